# revision 1
# baseline (speedup 1.0000x reference)
"""Trainium2 Bass kernel for NarrativeClassificationLoss.

Data-parallel over batch: each of 8 cores processes a 2048-row shard and
emits per-class partial sums packed into 3 DRAM tensors; the host
combines them in float64 (the pos_weight "all-reduce" over the batch
happens at gather time).

Software-pipelined, engine-balanced design (per-core busy: DVE ~58us,
ACT ~56us, DMA 54us, Pool ~42us, PE ~42us):
  Stage A (mega k):   Pool sfn = -y neg-cast (i32->bf16, dtype-blind),
      ACT E = exp(-x), spn = ln(1+E) = softplus(-x),
      DVE u' = sfn*spn and the K=8 group-min tree over spn,
      PE S' = ones @ sfn.
  Stage B (emitted one mega later so no engine's queue blocks on a slow
      cross-engine producer):  spp = x + spn computed split by columns
      (Pool cols 0:spp_cols mixed f32+bf16 add; DVE the rest), DVE w' =
      sfn*spp, focal e split by columns (ACT exp(-2*spp) for cols
      0:e_cols; DVE (1-1/(1+E))^2 via reciprocal for the tail), then
      the C/D/FC matmuls of that mega.
  PE: C = (-nl)^T@(-y*spn) = +C, D' = (-nl)^T@(spp + (-y*spp)) = -D
      (two accumulating matmul pairs into one PSUM bank), S' = -S,
      AB = ones@[u_n|w_n|nln|spp_n], FC focal Frobenius accumulator.
Labels are cast NEGATED (sfn = -y, nln = -nl), which kills all (1-y)
tensor_scalar ops; the host combine flips the signs back.
Narrative p_n = sigmoid(x_n) is column-split: ACT exp(-spn) for the
first pn_cols classes, DVE reciprocal of 1+E_n for the rest (the split
breaks the 1x reciprocal's serial section); e8_n = (2*sqrt(2)*(1-p_n))^2.
Hierarchy: sigmoid(max_k x_sub) = exp(-min_k softplus(-x_sub)) — MIN
tree over spn; -relu(g) = min(-g, 0) folds the negated mask; row sums
accumulate via tensor_scalar accum_out.
Knobs in CONFIG were tuned by ~400-config TimelineSim sweeps down to
single-column granularity; defaults are the best measured configuration
(71446 ns/core vs 75132 ns baseline, rel err 2.9e-4).
The late hier-chunk triggers (hier_plan) matter: inserting hierarchy
work one mega later than its data dependency allows keeps the mega
pipeline from hiccuping mid-stream.
"""

import numpy as np

import concourse.bacc as bacc
import concourse.tile as tile
from concourse import mybir
from concourse.bass_utils import run_bass_kernel_spmd

B = 16384
NCORES = 8
BL = B // NCORES          # 2048 rows per core
NN = 128                  # narrative classes
NS = 1024                 # subnarrative classes
K = NS // NN              # 8 subnarratives per narrative
NT = BL // 128            # 16 batch tiles of 128 rows

f32 = mybir.dt.float32
bf16 = mybir.dt.bfloat16
i32 = mybir.dt.int32
AF = mybir.ActivationFunctionType
OP = mybir.AluOpType
SQRT8 = 2.8284271247461903

_CACHE = {}
CONFIG = {
    "megas": [(0, 1), (1, 1), (2, 2), (4, 2), (6, 2), (8, 2), (10, 2), (12, 2), (14, 1), (15, 1)],
    "dve_e_t0": set(),         # megas whose focal-e runs fully on DVE recip path
    "sf_mode": "pool",         # 'split' | 'pool' | set of t0 on DVE
    "hier_eng": "dve",         # 'pool' | 'dve' for gd/hm
    "sppn_eng": "dve",         # narr spp engine
    "ss_evac": "act",          # 'dve' | 'act'
    "last_b": False,           # B-structure tail for last mega
    "spp_mode": "split",       # 'pool' | 'dve' | 'split'
    "lag": 1,                  # stage_b lag in megas
    "bufs": {"xs": 4, "ls": 4, "spn": 4, "Et": 3, "sf": 3, "u": 3, "spp": 2, "w": 2, "e": 2},
    "narr_recip": True,        # p_n/e8_n via DVE reciprocal (else ACT exp)
    "spp_cols": 552,           # columns of spp on Pool when spp_mode=='split'
    "sf_cols": 512,            # columns of sf on Pool when sf_mode=='split'
    "e_cols": 924,             # columns of focal-e on ACT; rest via DVE recip
    "evac_d": "act",           # 'dve' | 'act' for D/F/AB evacuation
    "nln_eng": "dve",          # narrative label cast engine
    "unwn_eng": "dve",         # u_n/w_n engine
    "e8_act": False,           # e8_n via ACT exp (p_n still DVE recip)
    "hier2": False,            # fold tile 15 into hier chunk 1
    "dma_small_first": False,  # emit small output DMAs before o_cd
    "narr_after": 1,           # number of megas loaded before narrative DMA
    "m1_pool_n": 0,            # narrative-class slice of tree level 1 on Pool
    "pn_act": False,           # p_n via ACT exp(-spn) instead of DVE recip
    "xnb_pool": False,         # narr logits bf16 cast on Pool; spp_n all-bf16
    "split0": False,           # halve mega-0 loads+stageA by columns (fast ramp)
    "narr_pos": 1,             # emit narr chain after this many stage_a megas
    # hier chunk plan: list of (col, h0, hn, trigger_pt); trigger_pt = t0+nt of
    # the stage_b mega after which the chunk is emitted (99 = end)
    "hier_plan": [(0, 0, 8, 10), (1, 8, 7, 16), (2, 15, 1, 99)],
    "abss_early": False,       # evac+DMA o_abss right after the stream loop
    "narr_mm_pos": 1,          # emit narr AB/FC matmuls after this many stage_a megas
    "spp_cols_map": {},        # per-mega t0 -> spp_cols override
    "lt_first": False,         # load labels before logits per mega
    "e_cols_map": {},          # per-mega t0 -> e_cols override
    "pn_cols": 32,             # narr p_n classes on ACT exp; rest DVE recip
}
LAST_RESULT = None

_ACT_SET = "natural_log_exp_and_others"


def _pin_act_tables(nc):
    """Thin the activation-table map so every func we use resolves to the
    one set that contains exp+ln+copy+relu — a single table load instead
    of thrashing between exp_and_others and natural_log_exp_and_others."""
    from concourse.hw_specs import get_activation_tables

    tabs = get_activation_tables(nc.m.arch)  # functools.cache'd: mutate in place
    ours = {AF.Exp, AF.Ln, AF.Copy, AF.Relu, AF.Identity}
    if _ACT_SET in tabs and ours - {AF.Identity} <= tabs[_ACT_SET]:
        for name, s in tabs.items():
            if name != _ACT_SET:
                s -= ours


def _build(reps=1):
    nc = bacc.Bacc()
    _pin_act_tables(nc)

    xn = nc.declare_dram_parameter("narrative_logits", [BL, NN], f32, isOutput=False)
    xs = nc.declare_dram_parameter("subnarrative_logits", [BL, NS], f32, isOutput=False)
    yn = nc.declare_dram_parameter("narrative_labels", [BL, NN], i32, isOutput=False)
    ys = nc.declare_dram_parameter("subnarrative_labels", [BL, NS], i32, isOutput=False)

    o_cd = nc.declare_dram_parameter("o_cd", [NN, 2 * NS], bf16, isOutput=True)
    o_fh = nc.declare_dram_parameter("o_fh", [NN, NN + 4], f32, isOutput=True)
    o_abss = nc.declare_dram_parameter("o_abss", [1, 512 + NS], f32, isOutput=True)

    with tile.TileContext(nc) as tc:
        with (
            tc.tile_pool(name="persist", bufs=1) as P1,
            tc.tile_pool(name="stream", bufs=2) as ST,
            tc.tile_pool(name="psum", bufs=1, space="PSUM") as PS,
        ):
            ones = P1.tile([128, 1], bf16)
            nc.vector.memset(ones, 1.0)

            for _rep in range(reps):
                _emit(nc, P1, ST, PS, ones, xn, xs, yn, ys,
                      o_cd, o_fh, o_abss)

    nc.finalize()
    return nc


def _emit(nc, P1, ST, PS, ones, xn, xs, yn, ys, o_cd, o_fh, o_abss):
    # persistent slabs: nar4 packs [u_n | w_n | nln | spp_n] for one AB matmul
    nar4 = P1.tile([128, NT, 512], bf16)
    u_n = nar4[:, :, 0:128]
    w_n = nar4[:, :, 128:256]
    nln = nar4[:, :, 256:384]                # NEGATED narrative labels (-nl)
    spp_n = nar4[:, :, 384:512]
    p_n = P1.tile([128, NT, NN], bf16)       # sigmoid(narr logits) via recip
    e8_n = P1.tile([128, NT, NN], bf16)      # 8*(1-p_n)^2
    # group-MIN of softplus(-x_sub): softplus(-max_k x) = min_k softplus(-x_k)
    sm_all = P1.tile([128, NT, NN], bf16)
    fh = P1.tile([128, NN + 4], f32)
    hacc = fh[:, NN : NN + 4]
    nc.vector.memset(hacc, 0.0)

    # PSUM accumulators (8 banks exactly)
    C0 = PS.tile([128, 512], f32, tag="C0")
    C1 = PS.tile([128, 512], f32, tag="C1")
    D0 = PS.tile([128, 512], f32, tag="D0")
    D1 = PS.tile([128, 512], f32, tag="D1")
    S0 = PS.tile([1, 512], f32, tag="S0")
    S1 = PS.tile([1, 512], f32, tag="S1")
    AB = PS.tile([1, 512], f32, tag="AB")
    FC = PS.tile([128, 128], f32, tag="FC")

    xsr = xs[:, :].rearrange("(m q p) c -> m p q c", q=16, p=128)[0]  # [128,16,1024]
    ysr = ys[:, :].rearrange("(m q p) c -> m p q c", q=16, p=128)[0]
    xnr = xn[:, :].rearrange("(t p) c -> p t c", p=128)
    ynr = yn[:, :].rearrange("(t p) c -> p t c", p=128)

    def load_mega(t0, nt):
        xt = ST.tile([128, nt, NS], f32, tag="xs", bufs=CONFIG["bufs"]["xs"])
        lt = ST.tile([128, nt, NS], i32, tag="ls", bufs=CONFIG["bufs"]["ls"])
        if t0 == 0 and nt == 1 and CONFIG["split0"]:
            # column-halved first loads: compute can start on cols 0:512
            # ~1.5us earlier, which shifts the whole (gap-free) DVE
            # timeline left
            nc.sync.dma_start(out=lt[:, :, 0:512], in_=ysr[:, 0:1, 0:512])
            nc.sync.dma_start(out=xt[:, :, 0:512], in_=xsr[:, 0:1, 0:512])
            nc.sync.dma_start(out=lt[:, :, 512:1024], in_=ysr[:, 0:1, 512:1024])
            nc.sync.dma_start(out=xt[:, :, 512:1024], in_=xsr[:, 0:1, 512:1024])
        elif CONFIG["lt_first"]:
            nc.sync.dma_start(out=lt, in_=ysr[:, t0 : t0 + nt, :])
            nc.sync.dma_start(out=xt, in_=xsr[:, t0 : t0 + nt, :])
        else:
            nc.sync.dma_start(out=xt, in_=xsr[:, t0 : t0 + nt, :])
            nc.sync.dma_start(out=lt, in_=ysr[:, t0 : t0 + nt, :])
        return xt, lt

    def stage_a0_halves(xt, lt):
        # mega (0,1) stage A in column halves for a faster pipeline ramp
        sf = ST.tile([128, 1, NS], bf16, tag="sf", bufs=CONFIG["bufs"]["sf"])
        Et = ST.tile([128, 1, NS], bf16, tag="Et", bufs=CONFIG["bufs"]["Et"])
        spn = ST.tile([128, 1, NS], bf16, tag="spn", bufs=CONFIG["bufs"]["spn"])
        u = ST.tile([128, 1, NS], bf16, tag="u", bufs=CONFIG["bufs"]["u"])
        sg = spn.rearrange("p q (n k) -> p q n k", k=K)
        m1 = ST.tile([128, 1, NN, 4], bf16, tag="m1", bufs=1)
        m2 = ST.tile([128, 1, NN, 2], bf16, tag="m2", bufs=1)
        for hi, (cl, ch) in enumerate(((0, 512), (512, 1024))):
            nl, nh = cl // K, ch // K
            nc.gpsimd.tensor_scalar(sf[:, :, cl:ch], lt[:, :, cl:ch], -1.0, 0.0,
                                    op0=OP.mult, op1=OP.add)
            nc.scalar.activation(Et[:, :, cl:ch], xt[:, :, cl:ch], AF.Exp, scale=-1.0)
            nc.scalar.activation(spn[:, :, cl:ch], Et[:, :, cl:ch], AF.Ln, bias=1.0)
            nc.vector.tensor_mul(u[:, :, cl:ch], sf[:, :, cl:ch], spn[:, :, cl:ch])
            nc.vector.tensor_tensor(m1[:, :, nl:nh, :], sg[:, :, nl:nh, 0:4],
                                    sg[:, :, nl:nh, 4:8], op=OP.min)
            nc.vector.tensor_tensor(m2[:, :, nl:nh, :], m1[:, :, nl:nh, 0:2],
                                    m1[:, :, nl:nh, 2:4], op=OP.min)
            nc.vector.tensor_tensor(sm_all[:, 0:1, nl:nh], m2[:, :, nl:nh, 0],
                                    m2[:, :, nl:nh, 1], op=OP.min)
            Sb = S0 if hi == 0 else S1
            nc.tensor.matmul(Sb, ones, sf[:, 0, cl:ch], start=True, stop=False)
        return (0, 1, xt, sf, Et, spn, u)

    def stage_a(t0, nt, xt, lt):
        # negated label cast (-y); engine split per CONFIG
        sf = ST.tile([128, nt, NS], bf16, tag="sf", bufs=CONFIG["bufs"]["sf"])
        m = CONFIG["sf_mode"]
        if m == "split":
            sc = CONFIG["sf_cols"]
            nc.gpsimd.tensor_scalar(sf[:, :, 0:sc], lt[:, :, 0:sc], -1.0, 0.0,
                                    op0=OP.mult, op1=OP.add)
            nc.vector.tensor_scalar(sf[:, :, sc:1024], lt[:, :, sc:1024], -1.0, 0.0,
                                    op0=OP.mult, op1=OP.add)
        else:
            eng = nc.vector if (isinstance(m, set) and t0 in m) else nc.gpsimd
            eng.tensor_scalar(sf, lt, -1.0, 0.0, op0=OP.mult, op1=OP.add)

        Et = ST.tile([128, nt, NS], bf16, tag="Et", bufs=CONFIG["bufs"]["Et"])
        nc.scalar.activation(Et, xt, AF.Exp, scale=-1.0)
        spn = ST.tile([128, nt, NS], bf16, tag="spn", bufs=CONFIG["bufs"]["spn"])
        nc.scalar.activation(spn, Et, AF.Ln, bias=1.0)

        u = ST.tile([128, nt, NS], bf16, tag="u", bufs=CONFIG["bufs"]["u"])
        nc.vector.tensor_mul(u, sf, spn)              # -y*spn

        # grouped MIN of spn over K=8 via pairwise tree
        sg = spn.rearrange("p q (n k) -> p q n k", k=K)
        m1 = ST.tile([128, nt, NN, 4], bf16, tag="m1", bufs=1)
        mp = CONFIG["m1_pool_n"]
        if mp > 0:
            nc.gpsimd.tensor_tensor(m1[:, :, 0:mp, :], sg[:, :, 0:mp, 0:4],
                                    sg[:, :, 0:mp, 4:8], op=OP.min)
            nc.vector.tensor_tensor(m1[:, :, mp:NN, :], sg[:, :, mp:NN, 0:4],
                                    sg[:, :, mp:NN, 4:8], op=OP.min)
        else:
            nc.vector.tensor_tensor(m1, sg[:, :, :, 0:4], sg[:, :, :, 4:8], op=OP.min)
        m2 = ST.tile([128, nt, NN, 2], bf16, tag="m2", bufs=1)
        nc.vector.tensor_tensor(m2, m1[:, :, :, 0:2], m1[:, :, :, 2:4], op=OP.min)
        nc.vector.tensor_tensor(
            sm_all[:, t0 : t0 + nt, :], m2[:, :, :, 0], m2[:, :, :, 1], op=OP.min
        )
        for q in range(nt):
            t = t0 + q
            s0f = (t == 0) and not CONFIG["split0"]
            nc.tensor.matmul(S0, ones, sf[:, q, 0:512], start=s0f, stop=(t == NT - 1))
            nc.tensor.matmul(S1, ones, sf[:, q, 512:1024], start=s0f, stop=(t == NT - 1))
        return (t0, nt, xt, sf, Et, spn, u)

    def stage_b(st, last=False):
        t0, nt, xt, sf, Et, spn, u = st
        if last:
            # tail path: xb cast starts at DMA (no wait on spn), and the
            # D accumulation uses nl^T@(xb + spn + v + u) = nl^T@((1-y)*spp)
            xb = ST.tile([128, nt, NS], bf16, tag="spp", bufs=CONFIG["bufs"]["spp"])
            nc.gpsimd.tensor_copy(out=xb, in_=xt)
            v = ST.tile([128, nt, NS], bf16, tag="w", bufs=CONFIG["bufs"]["w"])
            nc.vector.tensor_mul(v, sf, xb)           # -y*x
        else:
            # spp = x + softplus(-x) = softplus(x), mixed f32+bf16 add
            spp = ST.tile([128, nt, NS], bf16, tag="spp", bufs=CONFIG["bufs"]["spp"])
            sm = CONFIG["spp_mode"]
            if sm == "pool":
                nc.gpsimd.tensor_tensor(spp, xt, spn, op=OP.add)
            elif sm == "dve":
                nc.vector.tensor_add(spp, xt, spn)
            else:
                pc = CONFIG["spp_cols_map"].get(t0, CONFIG["spp_cols"])
                nc.gpsimd.tensor_tensor(spp[:, :, 0:pc], xt[:, :, 0:pc],
                                        spn[:, :, 0:pc], op=OP.add)
                nc.vector.tensor_add(spp[:, :, pc:1024], xt[:, :, pc:1024],
                                     spn[:, :, pc:1024])
            w = ST.tile([128, nt, NS], bf16, tag="w", bufs=CONFIG["bufs"]["w"])
            nc.vector.tensor_mul(w, sf, spp)          # -y*spp

        e = ST.tile([128, nt, NS], bf16, tag="e", bufs=CONFIG["bufs"]["e"])
        ec = CONFIG["e_cols_map"].get(t0, CONFIG["e_cols"])
        if t0 in CONFIG["dve_e_t0"]:
            ec = 0
        if ec > 0:
            nc.scalar.activation(e[:, :, 0:ec], spp[:, :, 0:ec], AF.Exp, scale=-2.0)
        if ec < NS:
            # focal e tail on DVE: e = (1 - 1/(1+E))^2 = (1-sigmoid(x))^2
            tt = ST.tile([128, nt, NS], bf16, tag="tt", bufs=1)
            nc.vector.tensor_scalar(tt[:, :, ec:NS], Et[:, :, ec:NS], 1.0, 1.0,
                                    op0=OP.mult, op1=OP.add)
            rr = ST.tile([128, nt, NS], bf16, tag="rr", bufs=1)
            with nc.allow_low_precision("0.1-weighted focal term; 2e-2 budget"):
                nc.vector.reciprocal(rr[:, :, ec:NS], tt[:, :, ec:NS])
            dd = ST.tile([128, nt, NS], bf16, tag="dd", bufs=1)
            nc.vector.tensor_scalar(dd[:, :, ec:NS], rr[:, :, ec:NS], -1.0, 1.0,
                                    op0=OP.mult, op1=OP.add)
            nc.vector.tensor_mul(e[:, :, ec:NS], dd[:, :, ec:NS], dd[:, :, ec:NS])

        for q in range(nt):
            t = t0 + q
            stt = t == 0
            sp = t == NT - 1
            nlT = nln[:, t, :]
            nc.tensor.matmul(C0, nlT, u[:, q, 0:512], start=stt, stop=sp)
            nc.tensor.matmul(C1, nlT, u[:, q, 512:1024], start=stt, stop=sp)
            if last:
                for rhs in (xb, spn):
                    nc.tensor.matmul(D0, nlT, rhs[:, q, 0:512], start=False, stop=False)
                    nc.tensor.matmul(D1, nlT, rhs[:, q, 512:1024], start=False, stop=False)
                nc.tensor.matmul(D0, nlT, v[:, q, 0:512], start=False, stop=False)
                nc.tensor.matmul(D0, nlT, u[:, q, 0:512], start=False, stop=sp)
                nc.tensor.matmul(D1, nlT, v[:, q, 512:1024], start=False, stop=False)
                nc.tensor.matmul(D1, nlT, u[:, q, 512:1024], start=False, stop=sp)
            else:
                nc.tensor.matmul(D0, nlT, spp[:, q, 0:512], start=stt, stop=False)
                nc.tensor.matmul(D1, nlT, spp[:, q, 512:1024], start=stt, stop=False)
                nc.tensor.matmul(D0, nlT, w[:, q, 0:512], start=False, stop=sp)
                nc.tensor.matmul(D1, nlT, w[:, q, 512:1024], start=False, stop=sp)
            for j in range(K):
                nc.tensor.matmul(
                    FC,
                    e[:, q, j * 128 : (j + 1) * 128],
                    u[:, q, j * 128 : (j + 1) * 128],
                    start=((CONFIG["narr_pos"] != 1 or CONFIG["narr_mm_pos"] != 1)
                           and t == 0 and j == 0),
                    stop=(t == NT - 1 and j == K - 1),
                )

    def hier_chunk(ci, h0, hn):
        pg = ST.tile([128, hn, NN], bf16, tag="pg", bufs=1)
        nc.scalar.activation(pg, sm_all[:, h0 : h0 + hn, :], AF.Exp, scale=-1.0)
        heng = nc.gpsimd if CONFIG["hier_eng"] == "pool" else nc.vector
        gd = ST.tile([128, hn, NN], bf16, tag="gd", bufs=1)
        heng.tensor_sub(gd, pg, p_n[:, h0 : h0 + hn, :])
        hm = ST.tile([128, hn, NN], bf16, tag="hm", bufs=1)
        heng.tensor_mul(hm, gd, nln[:, h0 : h0 + hn, :])   # -(pg-pn)*nl
        hr = ST.tile([128, hn, NN], bf16, tag="hr", bufs=1)
        # -relu(g) = min(-g, 0); accumulate row sums (negated hier loss)
        nc.vector.tensor_scalar(hr, hm, 0.0, 0.0, op0=OP.min, op1=OP.add,
                                accum_out=hacc[:, ci : ci + 1])

    # ---- fill: mega0 DMA first (feeds ACT earliest), then narrative DMA;
    # mega0 stage A before the narrative chain so ACT never waits.
    MEGAS = CONFIG["megas"]
    NA = CONFIG["narr_after"]
    early = []
    for t0, nt in MEGAS[:NA]:
        early.append((t0, nt) + load_mega(t0, nt))
    xn_sb = ST.tile([128, NT, NN], f32, tag="xs", bufs=CONFIG["bufs"]["xs"])
    yn_sb = ST.tile([128, NT, NN], i32, tag="ls", bufs=CONFIG["bufs"]["ls"])
    nc.sync.dma_start(out=xn_sb, in_=xnr)
    nc.sync.dma_start(out=yn_sb, in_=ynr)

    if CONFIG["split0"] and early[0][0] == 0 and early[0][1] == 1:
        pend = stage_a0_halves(early[0][2], early[0][3])
    else:
        pend = stage_a(*early[0])
    early_rest = early[1:]

    # ---------------- narrative chain ----------------
    def narr_chain():
      nln_eng = nc.gpsimd if CONFIG["nln_eng"] == "pool" else nc.vector
      nln_eng.tensor_scalar(nln, yn_sb, -1.0, 0.0, op0=OP.mult, op1=OP.add)
      E_n = ST.tile([128, NT, NN], bf16, tag="Et", bufs=CONFIG["bufs"]["Et"])
      nc.scalar.activation(E_n, xn_sb, AF.Exp, scale=-1.0)
      spn_n = ST.tile([128, NT, NN], bf16, tag="spn", bufs=CONFIG["bufs"]["spn"])
      nc.scalar.activation(spn_n, E_n, AF.Ln, bias=1.0)

      if CONFIG["narr_recip"]:
          # p_n = sigmoid(x_n): ACT exp(-spn) or DVE 1/(1+E_n);
          # e8_n = (2*sqrt(2)*(1-p_n))^2
          pc = CONFIG["pn_cols"]
          if CONFIG["pn_act"]:
              nc.scalar.activation(p_n, spn_n, AF.Exp, scale=-1.0)
          else:
              if pc > 0:
                  nc.scalar.activation(p_n[:, :, 0:pc], spn_n[:, :, 0:pc],
                                       AF.Exp, scale=-1.0)
              t_n = ST.tile([128, NT, NN], bf16, tag="tt", bufs=1)
              nc.vector.tensor_scalar(t_n[:, :, pc:NN], E_n[:, :, pc:NN],
                                      1.0, 1.0, op0=OP.mult, op1=OP.add)
              with nc.allow_low_precision("sigmoid for hier/focal; 2e-2 budget"):
                  nc.vector.reciprocal(p_n[:, :, pc:NN], t_n[:, :, pc:NN])
          if not CONFIG["e8_act"]:
              d8_n = ST.tile([128, NT, NN], bf16, tag="dd", bufs=1)
              nc.vector.tensor_scalar(d8_n, p_n, -SQRT8, SQRT8, op0=OP.mult, op1=OP.add)
              nc.vector.tensor_mul(e8_n, d8_n, d8_n)

      if CONFIG["xnb_pool"]:
          xnb = ST.tile([128, NT, NN], bf16, tag="w")
          nc.gpsimd.tensor_copy(out=xnb, in_=xn_sb)
          nc.vector.tensor_add(spp_n, xnb, spn_n)
      elif CONFIG["sppn_eng"] == "pool":
          nc.gpsimd.tensor_tensor(spp_n, xn_sb, spn_n, op=OP.add)
      else:
          nc.vector.tensor_add(spp_n, xn_sb, spn_n)
      if not CONFIG["narr_recip"]:
          nc.scalar.activation(p_n, spn_n, AF.Exp, scale=-1.0)
          ln8 = P1.tile([128, 1], f32)
          nc.vector.memset(ln8, 2.0794415416798357)
          nc.scalar.activation(e8_n, spp_n, AF.Exp, scale=-2.0, bias=ln8)
      unwn = nc.gpsimd if CONFIG["unwn_eng"] == "pool" else nc.vector
      unwn.tensor_mul(u_n, nln, spn_n)       # -nl*spn
      unwn.tensor_mul(w_n, nln, spp_n)       # -nl*spp
      if CONFIG["narr_recip"] and CONFIG["e8_act"]:
          ln8 = P1.tile([128, 1], f32)
          nc.vector.memset(ln8, 2.0794415416798357)
          nc.scalar.activation(e8_n, spp_n, AF.Exp, scale=-2.0, bias=ln8)

      if CONFIG["narr_mm_pos"] <= CONFIG["narr_pos"]:
          narr_matmuls()

    ASS_sb = P1.tile([1, 512 + NS], f32)

    def emit_abss():
        nc.vector.tensor_copy(ASS_sb[:, 0:512], AB)
        if CONFIG["ss_evac"] == "act":
            nc.scalar.copy(ASS_sb[:, 512:1024], S0)
            nc.scalar.copy(ASS_sb[:, 1024:1536], S1)
        else:
            nc.vector.tensor_copy(ASS_sb[:, 512:1024], S0)
            nc.vector.tensor_copy(ASS_sb[:, 1024:1536], S1)
        if CONFIG["abss_early"]:
            nc.sync.dma_start(out=o_abss[:, :], in_=ASS_sb)

    def narr_matmuls():
        first_fc = CONFIG["narr_pos"] == 1 and CONFIG["narr_mm_pos"] == 1
        for t in range(NT):
            nc.tensor.matmul(AB, ones, nar4[:, t, :],
                             start=(t == 0), stop=(t == NT - 1))
            nc.tensor.matmul(FC, e8_n[:, t, :], u_n[:, t, :],
                             start=(t == 0 and first_fc), stop=False)

    # ---------------- pipelined sub stream ----------------
    from collections import deque
    q = deque([pend])
    LAG = CONFIG["lag"]
    n_stage_a = 1
    if CONFIG["narr_pos"] <= 1:
        narr_chain()
    def flush_one(last=False):
        st = q.popleft()
        stage_b(st, last=last)
        pt = st[0] + st[1]
        for ci, h0, hn, tp in CONFIG["hier_plan"]:
            if tp == pt:
                hier_chunk(ci, h0, hn)
    for item in early_rest:
        q.append(stage_a(*item))
        if len(q) > LAG:
            flush_one()
    for t0, nt in MEGAS[NA:]:
        xt, lt = load_mega(t0, nt)
        q.append(stage_a(t0, nt, xt, lt))
        n_stage_a += 1
        if n_stage_a == CONFIG["narr_pos"]:
            narr_chain()
        if n_stage_a == CONFIG["narr_mm_pos"] and CONFIG["narr_mm_pos"] > CONFIG["narr_pos"]:
            narr_matmuls()
        if CONFIG["hier2"] and t0 + nt == 16:
            hier_chunk(1, 8, 8)
        if len(q) > LAG:
            flush_one()
    while len(q) > 1:
        flush_one()
    flush_one(last=CONFIG["last_b"])
    if CONFIG["abss_early"]:
        emit_abss()
    for ci, h0, hn, tp in CONFIG["hier_plan"]:
        if tp == 99:
            hier_chunk(ci, h0, hn)

    # ---------------- evacuate + store ----------------
    CD_sb = P1.tile([128, 2 * NS], bf16)
    nc.scalar.copy(CD_sb[:, 0:512], C0)
    nc.scalar.copy(CD_sb[:, 512:1024], C1)
    if CONFIG["evac_d"] == "act":
        nc.scalar.copy(CD_sb[:, 1024:1536], D0)
        nc.vector.tensor_copy(CD_sb[:, 1536:2048], D1)
    else:
        nc.vector.tensor_copy(CD_sb[:, 1024:1536], D0)
        nc.vector.tensor_copy(CD_sb[:, 1536:2048], D1)
    nc.vector.tensor_copy(fh[:, 0:NN], FC)
    if not CONFIG["abss_early"]:
        emit_abss()

    if CONFIG["dma_small_first"]:
        nc.sync.dma_start(out=o_abss[:, :], in_=ASS_sb)
        nc.sync.dma_start(out=o_fh[:, :], in_=fh)
        nc.sync.dma_start(out=o_cd[:, :], in_=CD_sb)
    else:
        nc.sync.dma_start(out=o_cd[:, :], in_=CD_sb)
        nc.sync.dma_start(out=o_fh[:, :], in_=fh)
        nc.sync.dma_start(out=o_abss[:, :], in_=ASS_sb)


def kernel(
    narrative_logits, subnarrative_logits, narrative_labels, subnarrative_labels
):
    global LAST_RESULT
    if "nc" not in _CACHE:
        _CACHE["nc"] = _build()
    nc = _CACHE["nc"]

    in_maps = []
    for i in range(NCORES):
        s = slice(i * BL, (i + 1) * BL)
        in_maps.append(
            {
                "narrative_logits": np.ascontiguousarray(narrative_logits[s]),
                "subnarrative_logits": np.ascontiguousarray(subnarrative_logits[s]),
                "narrative_labels": np.ascontiguousarray(narrative_labels[s]),
                "subnarrative_labels": np.ascontiguousarray(subnarrative_labels[s]),
            }
        )

    res = run_bass_kernel_spmd(nc, in_maps, list(range(NCORES)))
    LAST_RESULT = res

    # ------- host combine (the batch "all-reduce"); flips the neg-label signs
    Af = np.zeros(NN, np.float64)
    Bn = np.zeros(NN, np.float64)
    Sn = np.zeros(NN, np.float64)
    Ss = np.zeros(NS, np.float64)
    Cf = np.zeros((NN, NS), np.float64)
    Df = np.zeros((NN, NS), np.float64)
    Ff = np.zeros((NN, NN), np.float64)
    H = 0.0
    for r in res.results:
        abss = r["o_abss"][0].astype(np.float64)
        ab = abss[0:512]
        Af += -ab[0:128]                      # A = sum y*spn   (u_n = -nl*spn)
        Bn += ab[384:512] + ab[128:256]       # Bn = sum spp - sum nl*spp
        Sn += -ab[256:384]                    # Sn = sum nl     (packed -nl)
        Ss += -abss[512:1536]
        cd = r["o_cd"].astype(np.float64)
        Cf += cd[:, 0:NS]                     # (-nl)^T @ (-y*spn) = +C
        Df += -cd[:, NS : 2 * NS]             # (-nl)^T @ (spp - y*spp) = -D
        fh = r["o_fh"].astype(np.float64)
        Ff += -fh[:, 0:NN]                    # e^T @ (-u) = -focal
        H += -fh[:, NN : NN + 4].sum()

    cc = np.arange(NS)
    Cd = Cf[cc // K, cc]
    Dd = Df[cc // K, cc]

    npw = np.clip((B - Sn) / (Sn + 1e-6), 1.0, 50.0)
    spw = np.clip((B - Ss) / (Ss + 1e-6), 1.0, 50.0)

    narrative_loss = (npw * Af + Bn).sum() / (B * NN)

    valid = Sn.sum()
    sub_loss = (spw * Cd + Dd).sum() / K / max(valid, 1.0) if valid > 0 else 0.0

    hier = H / B
    focal = 0.1 * np.trace(Ff) / (B * NS)

    total = narrative_loss + sub_loss + 0.5 * hier + focal
    return np.asarray(total, dtype=np.float32)

